# revision 53
# baseline (speedup 1.0000x reference)
"""MoE expert-parallel MLP kernel for Trainium2 (8 NeuronCores).

Problem: x:(1,8,2048,2048) f32, wi:(8,2048,4096), wo:(8,4096,2048)
         out = gelu_exact(x @ wi) @ wo   (per expert)

Sharding: expert parallelism — core e handles expert e entirely. No
collectives. Per-core math (C=2048 tokens, H=2048 hidden, I=4096 inter):

  GEMM1 (Strassen-1): h1[I, C] = wi[H, I].T @ xT[H, C]
  gelu:  h1 = gelu(h1)                     (ScalarE, exact erf gelu)
  GEMM2 (Strassen-1): out[C, H] = h1[I, C].T @ wo[I, H]

BOTH GEMMs run one level of Strassen over 2x2 blocks, so the PE streams
7/8 of the plain rows for each (rel err ~7e-3 vs the 2e-2 gate; all
operands bf16 at 1 cyc/row). The input-side combination matrices are
formed on the HOST: wi-combos and xT-combos for GEMM1, wo-combos for
GEMM2. GEMM2's activation-side combos (L1..L7, combinations of the four
gelu'd h1 quadrants) are built incrementally at GEMM1 drain time — the
four quadrant tiles of a position exist simultaneously, so 5 cheap bf16
adds produce them — and this 7-matrix A-store REPLACES h1 entirely
(56 KiB/partition vs 64).

Phasing: the C/2-wide quadrant-column space runs in FOUR 256-wide
phases S (tokens S*256..+256 and 1024+S*256..+256), each G1-Strassen
then G2-Strassen. Everything stays SBUF-resident; wi-combo and wo-combo
tiles re-stream per phase (the kernel runs ~95% DMA-busy, so cross-
phase prefetches are threaded into each phase's slack).

Recombination per position obeys the HW rules (DVE reads at most one
PSUM operand; GPSIMD touches no PSUM; ACT pulls the two doubly-used
products): 2 ACT copies + 6 DVE adds + 2 Pool SBUF-subs, ordered so
PSUM banks free in allocation order. G1 drains through ACT gelu into
the A-store; G2 drains straight to the output DMA.

PSUM: 7 [128,256] products pack into halves of 4 banks per position,
two positions ping-pong the 8 banks throughout.
"""
import numpy as np
from contextlib import ExitStack

import ml_dtypes
import concourse.bass as bass
import concourse.tile as tile
from concourse import bacc, mybir
from concourse.bass_utils import run_bass_kernel_spmd

P = 128
C, H, I = 2048, 2048, 4096
E = 8
F32 = mybir.dt.float32
BF16 = mybir.dt.bfloat16

H2, I2, C2 = H // 2, I // 2, C // 2   # 1024, 2048, 1024
K8 = H2 // P       # 8 k-subtiles per G1 product
K16 = I2 // P      # 16 k-subtiles per G2 product
NQ = 256           # product free width (half bank)
N5 = 512
AL = mybir.AluOpType


def _build():
    nc = bacc.Bacc("TRN2", target_bir_lowering=False, debug=False, num_devices=E)
    # wa: pretiled G1 lhsT combos; row (p*16+io)*128+pp, col k*128+i2
    wa = nc.dram_tensor("wa", [7 * 16 * P, K8 * P], BF16, kind="ExternalInput").ap()
    # xb: G1 rhs combos [7*H2, C2] (natural; 512B runs at 256-col slices)
    xb = nc.dram_tensor("xb", [7 * H2, C2], BF16, kind="ExternalInput").ap()
    # wr: pretiled G2 rhs combos; row (p*4+hg)*128+pp, col ik*256+h2
    wr = nc.dram_tensor("wr", [7 * 4 * P, K16 * NQ], BF16, kind="ExternalInput").ap()
    out = nc.dram_tensor("out", [C, H], F32, kind="ExternalOutput").ap()

    GELU = mybir.ActivationFunctionType.Gelu

    with tile.TileContext(nc) as tc, ExitStack() as ctx:
        apool = ctx.enter_context(tc.tile_pool(name="astore", bufs=7))
        wapool = ctx.enter_context(tc.tile_pool(name="wa", bufs=14))
        xbpool = ctx.enter_context(tc.tile_pool(name="xb", bufs=8))
        wrpool = ctx.enter_context(tc.tile_pool(name="wr", bufs=8))
        stage = ctx.enter_context(tc.tile_pool(name="stage", bufs=10))
        opool = ctx.enter_context(tc.tile_pool(name="outs", bufs=4))
        psum = ctx.enter_context(tc.tile_pool(name="psum", bufs=8, space="PSUM"))

        wa_t = {}

        def load_wa(S, io, p):
            t = wapool.tile([P, K8, P], BF16, tag="wa", name=f"wa_{S}_{io}_{p}")
            nc.sync.dma_start(
                t[:],
                wa[(p * 16 + io) * P:(p * 16 + io + 1) * P, :]
                .rearrange("pp (k i) -> pp k i", k=K8))
            wa_t[(S, io, p)] = t

        xb_t = {}

        def load_xb(S, p):
            t = xbpool.tile([P, K8, NQ], BF16, tag="xb", name=f"xb_{S}_{p}")
            nc.sync.dma_start(
                t[:],
                xb[p * H2:(p + 1) * H2, S * NQ:(S + 1) * NQ]
                .rearrange("(k pp) c -> pp k c", pp=P))
            xb_t[(S, p)] = t

        wr_t = {}

        def load_wr(S, hg, p):
            t = wrpool.tile([P, K16, NQ], BF16, tag="wr", name=f"wr_{S}_{hg}_{p}")
            nc.sync.dma_start(
                t[:],
                wr[(p * 4 + hg) * P:(p * 4 + hg + 1) * P, :]
                .rearrange("pp (k h) -> pp k h", k=K16))
            wr_t[(S, hg, p)] = t

        def alloc_ms(kind, S, a, b):
            mt = [psum.tile([P, N5], F32, tag="mm", name=f"m{kind}_{S}_{a}_{b}_{j}")
                  for j in range(4)]
            return [mt[p // 2][:, (p % 2) * NQ:(p % 2 + 1) * NQ]
                    for p in range(7)]

        def combine(ms, nm, sink, tail=False):
            """Strassen output recombination into sink(t11,t12,t21,t22).
            Each op reads at most ONE PSUM operand; banks free in order.
            tail=True fronts the longest chain (t22) so the kernel's final
            stores pipeline earliest."""
            def st(x):
                return stage.tile([P, NQ], F32, tag="st", name=f"{x}_{nm}")
            u = st("u"); a = st("a"); x = st("x"); b_ = st("b")
            c_ = st("c"); d_ = st("d")
            t11 = st("t11"); t12 = st("t12")
            t21 = st("t21"); t22 = st("t22")
            nc.scalar.copy(u[:], ms[0])                   # M1 (ACT)
            nc.scalar.copy(x[:], ms[4])                   # M5 (ACT)
            if tail:
                nc.vector.scalar_tensor_tensor(
                    c_[:], ms[1], -1.0, u[:], AL.mult, AL.add)
                nc.vector.tensor_add(d_[:], c_[:], ms[2])
                nc.vector.tensor_add(t22[:], d_[:], ms[5])
                nc.vector.tensor_add(a[:], u[:], ms[3])
                nc.vector.tensor_add(t12[:], x[:], ms[2])
                nc.vector.tensor_add(b_[:], a[:], ms[6])
            else:
                nc.vector.tensor_add(a[:], u[:], ms[3])       # M1+M4
                nc.vector.scalar_tensor_tensor(
                    c_[:], ms[1], -1.0, u[:], AL.mult, AL.add)  # M1-M2
                nc.vector.tensor_add(b_[:], a[:], ms[6])      # M1+M4+M7
                nc.vector.tensor_add(t12[:], x[:], ms[2])     # M5+M3
                nc.vector.tensor_add(d_[:], c_[:], ms[2])     # +M3
                nc.vector.tensor_add(t22[:], d_[:], ms[5])    # +M6
            nc.gpsimd.tensor_sub(t21[:], a[:], c_[:])     # M2+M4 (SBUF only)
            nc.gpsimd.tensor_sub(t11[:], b_[:], x[:])     # SBUF only
            sink(t11, t12, t21, t22)

        # ---- ramp: phase-0 xb set + first wa block, paced pairs ----
        for p in range(7):
            load_xb(0, p)
            load_wa(0, 0, p)

        L = None
        for S in range(4):
            # ---------- GEMM1 Strassen quarter-phase ----------
            # A-store: L1..L7 [128, 16io, 256] bf16 (replaces h1)
            L = [apool.tile([P, 16, NQ], BF16, tag="astore", name=f"L_{S}_{q}")
                 for q in range(7)]
            for io in range(16):
                if io + 1 < 16:
                    for p in range(7):
                        load_wa(S, io + 1, p)
                # seed this phase's first G2 hg-group: tiles 0-1 came from
                # the previous G2 phase (or here for S=0), rest stream in
                # the G1 tail's DMA slack
                if S == 0 and io in (12, 13):
                    load_wr(0, 0, io - 12)
                if io == 13:
                    load_wr(S, 0, 2)
                elif io == 14:
                    load_wr(S, 0, 3)
                    load_wr(S, 0, 4)
                elif io == 15:
                    load_wr(S, 0, 5)
                    load_wr(S, 0, 6)
                ms = alloc_ms(1, S, io, 0)
                for p in range(7):
                    wt = wa_t[(S, io, p)]
                    xt = xb_t[(S, p)]
                    for k in range(K8):
                        nc.tensor.matmul(ms[p], wt[:, k, :], xt[:, k, :],
                                         start=(k == 0), stop=(k == K8 - 1))

                def g1_sink(t11, t12, t21, t22, S=S, io=io):
                    gA = stage.tile([P, NQ], BF16, tag="st", name=f"g12_{S}_{io}")
                    gB = stage.tile([P, NQ], BF16, tag="st", name=f"g21_{S}_{io}")
                    l3 = L[2][:, io, :]
                    l4 = L[3][:, io, :]
                    nc.scalar.activation(l3, t11[:], GELU)   # g11 -> L3
                    nc.scalar.activation(gA[:], t12[:], GELU)  # g12
                    nc.scalar.activation(gB[:], t21[:], GELU)  # g21
                    nc.scalar.activation(l4, t22[:], GELU)   # g22 -> L4
                    nc.gpsimd.tensor_add(L[0][:, io, :], l3, l4)      # L1
                    nc.vector.tensor_add(L[1][:, io, :], gA[:], l4)   # L2
                    nc.gpsimd.tensor_add(L[4][:, io, :], l3, gB[:])   # L5
                    nc.vector.tensor_sub(L[5][:, io, :], gA[:], l3)   # L6
                    nc.gpsimd.tensor_sub(L[6][:, io, :], gB[:], l4)   # L7

                combine(ms, f"1_{S}_{io}", g1_sink)

            # ---------- GEMM2 Strassen quarter-phase ----------
            for hg in range(4):
                for co2 in range(2):
                    # spread prefetches into this position's shadow
                    # cross-phase loads go EARLY (hg0/hg1) so the DMA
                    # backlog that builds through this phase lands on the
                    # slack-rich wr prefetches instead of the next phase's
                    # first operands
                    if co2 == 0:
                        for p in range(2, 6):
                            if hg + 1 < 4:
                                load_wr(S, hg + 1, p - 2)
                    else:
                        if hg + 1 < 4:
                            for p in range(4, 7):
                                load_wr(S, hg + 1, p)
                        if hg == 0 and S + 1 < 4:
                            for p in range(7):
                                load_xb(S + 1, p)
                        if hg == 1 and S + 1 < 4:
                            for p in range(7):
                                load_wa(S + 1, 0, p)
                        if hg == 2 and S + 1 < 4:
                            load_wr(S + 1, 0, 0)
                            load_wr(S + 1, 0, 1)
                    ms = alloc_ms(2, S, hg, co2)
                    # the phase's very first position consumes k descending
                    # so it doesn't wait on the freshest L-store rows
                    korder = (list(reversed(range(K16)))
                              if hg == 0 and co2 == 0 else list(range(K16)))
                    for p in range(7):
                        rt = wr_t[(S, hg, p)]
                        for ki, k in enumerate(korder):
                            nc.tensor.matmul(
                                ms[p], L[p][:, k, co2 * P:(co2 + 1) * P],
                                rt[:, k, :],
                                start=(ki == 0), stop=(ki == K16 - 1))

                    def g2_sink(t11, t12, t21, t22, S=S, hg=hg, co2=co2):
                        r0 = S * NQ + co2 * P          # C1 token rows
                        r1 = 1024 + S * NQ + co2 * P   # C2 token rows
                        h0 = hg * NQ                   # H1 cols
                        h1c = 1024 + hg * NQ           # H2 cols
                        last = (S == 3 and hg == 3 and co2 == 1)
                        if last:
                            # data-ready order, copies on idle engines so
                            # the final stores pipeline instead of queueing
                            # behind the DVE add chain
                            plan = ((t22, r1, h1c, nc.scalar),
                                    (t12, r0, h1c, nc.gpsimd),
                                    (t21, r1, h0, nc.gpsimd),
                                    (t11, r0, h0, nc.vector))
                        else:
                            plan = ((t11, r0, h0, nc.vector),
                                    (t12, r0, h1c, nc.vector),
                                    (t21, r1, h0, nc.vector),
                                    (t22, r1, h1c, nc.vector))
                        for t_, rr, hh, eng in plan:
                            ot = opool.tile([P, NQ], F32, tag="outs",
                                            name=f"o_{S}_{hg}_{co2}_{rr}_{hh}")
                            if eng is nc.scalar:
                                eng.copy(ot[:], t_[:])
                            else:
                                eng.tensor_copy(ot[:], t_[:])
                            nc.scalar.dma_start(
                                out[rr:rr + P, hh:hh + NQ], ot[:])

                    combine(ms, f"2_{S}_{hg}_{co2}", g2_sink,
                            tail=(S == 3 and hg == 3 and co2 == 1))

    nc.compile()
    return nc


_NC = None


def _host_prep(x, wi, wo):
    """Per-expert Strassen operand combos + bf16 casts (host side)."""
    bf = ml_dtypes.bfloat16
    xT = np.ascontiguousarray(np.swapaxes(x, 1, 2))      # [E, H, C]
    w11 = wi[:, :H2, :I2]; w12 = wi[:, :H2, I2:]
    w21 = wi[:, H2:, :I2]; w22 = wi[:, H2:, I2:]
    # G1 lhsT combos, product order M1..M7
    was = [w11 + w22, w12 + w22, w11, w22, w11 + w21, w12 - w11, w21 - w22]
    # pre-tile each combo [1024, 2048] -> [16io*128pp, 8k*128i2] (2KB runs)
    wa = np.stack(
        [np.ascontiguousarray(
            c.reshape(E, K8, P, 16, P)
            .transpose(0, 3, 2, 1, 4).reshape(E, 16 * P, K8 * P))
         for c in was], axis=1).reshape(E, 7 * 16 * P, K8 * P).astype(bf)
    b11 = xT[:, :H2, :C2]; b12 = xT[:, :H2, C2:]
    b21 = xT[:, H2:, :C2]; b22 = xT[:, H2:, C2:]
    xbs = [b11 + b22, b11, b12 - b22, b21 - b11, b22, b11 + b12, b21 + b22]
    xbc = np.concatenate(xbs, axis=1).astype(bf)         # [E, 7*H2, C2]
    # G2 rhs combos of wo quadrants, product order N1..N7
    q11 = wo[:, :I2, :H2]; q12 = wo[:, :I2, H2:]
    q21 = wo[:, I2:, :H2]; q22 = wo[:, I2:, H2:]
    wrs = [q11 + q22, q11, q12 - q22, q21 - q11, q22, q11 + q12, q21 + q22]
    # pre-tile [2048, 1024] -> [4hg*128pp, 16ik*256h2] (8KB runs)
    wrc = np.stack(
        [np.ascontiguousarray(
            c.reshape(E, K16, P, 4, NQ)
            .transpose(0, 3, 2, 1, 4).reshape(E, 4 * P, K16 * NQ))
         for c in wrs], axis=1).reshape(E, 7 * 4 * P, K16 * NQ).astype(bf)
    return wa, xbc, wrc


def kernel(x, wi, wo):
    global _NC
    if _NC is None:
        _NC = _build()
    x = np.asarray(x, dtype=np.float32).reshape(E, C, H)
    wi = np.ascontiguousarray(np.asarray(wi, dtype=np.float32))
    wo = np.ascontiguousarray(np.asarray(wo, dtype=np.float32))
    wa, xbc, wrc = _host_prep(x, wi, wo)
    in_maps = [{"wa": wa[e], "xb": xbc[e], "wr": wrc[e]} for e in range(E)]
    res = run_bass_kernel_spmd(_NC, in_maps, core_ids=list(range(E)))
    o = np.stack([res.results[e]["out"] for e in range(E)])[None]
    return o


# revision 54
# speedup vs baseline: 1.0002x; 1.0002x over previous
"""MoE expert-parallel MLP kernel for Trainium2 (8 NeuronCores).

Problem: x:(1,8,2048,2048) f32, wi:(8,2048,4096), wo:(8,4096,2048)
         out = gelu_exact(x @ wi) @ wo   (per expert)

Sharding: expert parallelism — core e handles expert e entirely. No
collectives. Per-core math (C=2048 tokens, H=2048 hidden, I=4096 inter):

  GEMM1 (Strassen-1): h1[I, C] = wi[H, I].T @ xT[H, C]
  gelu:  h1 = gelu(h1)                     (ScalarE, exact erf gelu)
  GEMM2 (Strassen-1): out[C, H] = h1[I, C].T @ wo[I, H]

BOTH GEMMs run one level of Strassen over 2x2 blocks, so the PE streams
7/8 of the plain rows for each (rel err ~7e-3 vs the 2e-2 gate; all
operands bf16 at 1 cyc/row). The input-side combination matrices are
formed on the HOST: wi-combos and xT-combos for GEMM1, wo-combos for
GEMM2. GEMM2's activation-side combos (L1..L7, combinations of the four
gelu'd h1 quadrants) are built incrementally at GEMM1 drain time — the
four quadrant tiles of a position exist simultaneously, so 5 cheap bf16
adds produce them — and this 7-matrix A-store REPLACES h1 entirely
(56 KiB/partition vs 64).

Phasing: the C/2-wide quadrant-column space runs in FOUR 256-wide
phases S (tokens S*256..+256 and 1024+S*256..+256), each G1-Strassen
then G2-Strassen. Everything stays SBUF-resident; wi-combo and wo-combo
tiles re-stream per phase (the kernel runs ~95% DMA-busy, so cross-
phase prefetches are threaded into each phase's slack).

Recombination per position obeys the HW rules (DVE reads at most one
PSUM operand; GPSIMD touches no PSUM; ACT pulls the two doubly-used
products): 2 ACT copies + 6 DVE adds + 2 Pool SBUF-subs, ordered so
PSUM banks free in allocation order. G1 drains through ACT gelu into
the A-store; G2 drains straight to the output DMA.

PSUM: 7 [128,256] products pack into halves of 4 banks per position,
two positions ping-pong the 8 banks throughout.
"""
import numpy as np
from contextlib import ExitStack

import ml_dtypes
import concourse.bass as bass
import concourse.tile as tile
from concourse import bacc, mybir
from concourse.bass_utils import run_bass_kernel_spmd

P = 128
C, H, I = 2048, 2048, 4096
E = 8
F32 = mybir.dt.float32
BF16 = mybir.dt.bfloat16

H2, I2, C2 = H // 2, I // 2, C // 2   # 1024, 2048, 1024
K8 = H2 // P       # 8 k-subtiles per G1 product
K16 = I2 // P      # 16 k-subtiles per G2 product
NQ = 256           # product free width (half bank)
N5 = 512
AL = mybir.AluOpType


def _build():
    nc = bacc.Bacc("TRN2", target_bir_lowering=False, debug=False, num_devices=E)
    # wa: pretiled G1 lhsT combos; row (p*16+io)*128+pp, col k*128+i2
    wa = nc.dram_tensor("wa", [7 * 16 * P, K8 * P], BF16, kind="ExternalInput").ap()
    # xb: G1 rhs combos [7*H2, C2] (natural; 512B runs at 256-col slices)
    xb = nc.dram_tensor("xb", [7 * H2, C2], BF16, kind="ExternalInput").ap()
    # wr: pretiled G2 rhs combos; row (p*4+hg)*128+pp, col ik*256+h2
    wr = nc.dram_tensor("wr", [7 * 4 * P, K16 * NQ], BF16, kind="ExternalInput").ap()
    out = nc.dram_tensor("out", [C, H], F32, kind="ExternalOutput").ap()

    GELU = mybir.ActivationFunctionType.Gelu

    with tile.TileContext(nc) as tc, ExitStack() as ctx:
        apool = ctx.enter_context(tc.tile_pool(name="astore", bufs=7))
        wapool = ctx.enter_context(tc.tile_pool(name="wa", bufs=14))
        xbpool = ctx.enter_context(tc.tile_pool(name="xb", bufs=8))
        wrpool = ctx.enter_context(tc.tile_pool(name="wr", bufs=8))
        stage = ctx.enter_context(tc.tile_pool(name="stage", bufs=10))
        opool = ctx.enter_context(tc.tile_pool(name="outs", bufs=4))
        psum = ctx.enter_context(tc.tile_pool(name="psum", bufs=8, space="PSUM"))

        wa_t = {}

        def load_wa(S, io, p):
            t = wapool.tile([P, K8, P], BF16, tag="wa", name=f"wa_{S}_{io}_{p}")
            nc.sync.dma_start(
                t[:],
                wa[(p * 16 + io) * P:(p * 16 + io + 1) * P, :]
                .rearrange("pp (k i) -> pp k i", k=K8))
            wa_t[(S, io, p)] = t

        xb_t = {}

        def load_xb(S, p):
            t = xbpool.tile([P, K8, NQ], BF16, tag="xb", name=f"xb_{S}_{p}")
            nc.sync.dma_start(
                t[:],
                xb[p * H2:(p + 1) * H2, S * NQ:(S + 1) * NQ]
                .rearrange("(k pp) c -> pp k c", pp=P))
            xb_t[(S, p)] = t

        wr_t = {}

        def load_wr(S, hg, p):
            t = wrpool.tile([P, K16, NQ], BF16, tag="wr", name=f"wr_{S}_{hg}_{p}")
            nc.sync.dma_start(
                t[:],
                wr[(p * 4 + hg) * P:(p * 4 + hg + 1) * P, :]
                .rearrange("pp (k h) -> pp k h", k=K16))
            wr_t[(S, hg, p)] = t

        def alloc_ms(kind, S, a, b):
            mt = [psum.tile([P, N5], F32, tag="mm", name=f"m{kind}_{S}_{a}_{b}_{j}")
                  for j in range(4)]
            return [mt[p // 2][:, (p % 2) * NQ:(p % 2 + 1) * NQ]
                    for p in range(7)]

        def combine(ms, nm, sink, tail=False):
            """Strassen output recombination into sink(t11,t12,t21,t22).
            Each op reads at most ONE PSUM operand; banks free in order.
            tail=True fronts the longest chain (t22) so the kernel's final
            stores pipeline earliest."""
            def st(x):
                return stage.tile([P, NQ], F32, tag="st", name=f"{x}_{nm}")
            u = st("u"); a = st("a"); x = st("x"); b_ = st("b")
            c_ = st("c"); d_ = st("d")
            t11 = st("t11"); t12 = st("t12")
            t21 = st("t21"); t22 = st("t22")
            nc.scalar.copy(u[:], ms[0])                   # M1 (ACT)
            nc.scalar.copy(x[:], ms[4])                   # M5 (ACT)
            if tail:
                nc.vector.scalar_tensor_tensor(
                    c_[:], ms[1], -1.0, u[:], AL.mult, AL.add)
                nc.vector.tensor_add(d_[:], c_[:], ms[2])
                nc.vector.tensor_add(t22[:], d_[:], ms[5])
                nc.vector.tensor_add(a[:], u[:], ms[3])
                nc.vector.tensor_add(t12[:], x[:], ms[2])
                nc.vector.tensor_add(b_[:], a[:], ms[6])
            else:
                nc.vector.tensor_add(a[:], u[:], ms[3])       # M1+M4
                nc.vector.scalar_tensor_tensor(
                    c_[:], ms[1], -1.0, u[:], AL.mult, AL.add)  # M1-M2
                nc.vector.tensor_add(b_[:], a[:], ms[6])      # M1+M4+M7
                nc.vector.tensor_add(t12[:], x[:], ms[2])     # M5+M3
                nc.vector.tensor_add(d_[:], c_[:], ms[2])     # +M3
                nc.vector.tensor_add(t22[:], d_[:], ms[5])    # +M6
            nc.gpsimd.tensor_sub(t21[:], a[:], c_[:])     # M2+M4 (SBUF only)
            nc.gpsimd.tensor_sub(t11[:], b_[:], x[:])     # SBUF only
            sink(t11, t12, t21, t22)

        # ---- ramp: phase-0 xb set + first wa block, paced pairs ----
        for p in range(7):
            load_xb(0, p)
            load_wa(0, 0, p)

        L = None
        for S in range(4):
            # ---------- GEMM1 Strassen quarter-phase ----------
            # A-store: L1..L7 [128, 16io, 256] bf16 (replaces h1)
            L = [apool.tile([P, 16, NQ], BF16, tag="astore", name=f"L_{S}_{q}")
                 for q in range(7)]
            for io in range(16):
                if io + 1 < 16:
                    for p in range(7):
                        load_wa(S, io + 1, p)
                # seed this phase's first G2 hg-group: tiles 0-1 came from
                # the previous G2 phase (or here for S=0), rest stream in
                # the G1 tail's DMA slack
                if S == 0 and io in (12, 13):
                    load_wr(0, 0, io - 12)
                if io == 13:
                    load_wr(S, 0, 2)
                elif io == 14:
                    load_wr(S, 0, 3)
                    load_wr(S, 0, 4)
                elif io == 15:
                    load_wr(S, 0, 5)
                    load_wr(S, 0, 6)
                ms = alloc_ms(1, S, io, 0)
                for p in range(7):
                    wt = wa_t[(S, io, p)]
                    xt = xb_t[(S, p)]
                    for k in range(K8):
                        nc.tensor.matmul(ms[p], wt[:, k, :], xt[:, k, :],
                                         start=(k == 0), stop=(k == K8 - 1))

                def g1_sink(t11, t12, t21, t22, S=S, io=io):
                    gA = stage.tile([P, NQ], BF16, tag="st", name=f"g12_{S}_{io}")
                    gB = stage.tile([P, NQ], BF16, tag="st", name=f"g21_{S}_{io}")
                    l3 = L[2][:, io, :]
                    l4 = L[3][:, io, :]
                    nc.scalar.activation(l3, t11[:], GELU)   # g11 -> L3
                    nc.scalar.activation(gA[:], t12[:], GELU)  # g12
                    nc.scalar.activation(gB[:], t21[:], GELU)  # g21
                    nc.scalar.activation(l4, t22[:], GELU)   # g22 -> L4
                    nc.gpsimd.tensor_add(L[0][:, io, :], l3, l4)      # L1
                    nc.vector.tensor_add(L[1][:, io, :], gA[:], l4)   # L2
                    nc.gpsimd.tensor_add(L[4][:, io, :], l3, gB[:])   # L5
                    nc.vector.tensor_sub(L[5][:, io, :], gA[:], l3)   # L6
                    nc.gpsimd.tensor_sub(L[6][:, io, :], gB[:], l4)   # L7

                combine(ms, f"1_{S}_{io}", g1_sink)

            # ---------- GEMM2 Strassen quarter-phase ----------
            for hg in range(4):
                for co2 in range(2):
                    # spread prefetches into this position's shadow
                    # cross-phase loads go EARLY (hg0/hg1) so the DMA
                    # backlog that builds through this phase lands on the
                    # slack-rich wr prefetches instead of the next phase's
                    # first operands
                    if co2 == 0:
                        for p in range(2, 6):
                            if hg + 1 < 4:
                                load_wr(S, hg + 1, p - 2)
                    else:
                        if hg + 1 < 4:
                            for p in range(4, 7):
                                load_wr(S, hg + 1, p)
                        if hg == 0 and S + 1 < 4:
                            for p in range(7):
                                load_xb(S + 1, p)
                        if hg == 1 and S + 1 < 4:
                            for p in range(7):
                                load_wa(S + 1, 0, p)
                        if hg == 2 and S + 1 < 4:
                            load_wr(S + 1, 0, 0)
                            load_wr(S + 1, 0, 1)
                    ms = alloc_ms(2, S, hg, co2)
                    # the phase's very first position consumes k descending
                    # so it doesn't wait on the freshest L-store rows
                    korder = (list(reversed(range(K16)))
                              if hg == 0 and co2 == 0 else list(range(K16)))
                    for p in range(7):
                        rt = wr_t[(S, hg, p)]
                        for ki, k in enumerate(korder):
                            nc.tensor.matmul(
                                ms[p], L[p][:, k, co2 * P:(co2 + 1) * P],
                                rt[:, k, :],
                                start=(ki == 0), stop=(ki == K16 - 1))

                    def g2_sink(t11, t12, t21, t22, S=S, hg=hg, co2=co2):
                        r0 = S * NQ + co2 * P          # C1 token rows
                        r1 = 1024 + S * NQ + co2 * P   # C2 token rows
                        h0 = hg * NQ                   # H1 cols
                        h1c = 1024 + hg * NQ           # H2 cols
                        last = (S == 3 and hg == 3 and co2 == 1)
                        if last:
                            # data-ready order, copies on idle engines and
                            # stores split across both DMA queues so the
                            # final issue latencies overlap
                            plan = ((t22, r1, h1c, nc.scalar, nc.scalar),
                                    (t12, r0, h1c, nc.gpsimd, nc.sync),
                                    (t21, r1, h0, nc.gpsimd, nc.scalar),
                                    (t11, r0, h0, nc.vector, nc.sync))
                        else:
                            plan = ((t11, r0, h0, nc.vector, nc.scalar),
                                    (t12, r0, h1c, nc.vector, nc.scalar),
                                    (t21, r1, h0, nc.vector, nc.scalar),
                                    (t22, r1, h1c, nc.vector, nc.scalar))
                        for t_, rr, hh, eng, dq in plan:
                            ot = opool.tile([P, NQ], F32, tag="outs",
                                            name=f"o_{S}_{hg}_{co2}_{rr}_{hh}")
                            if eng is nc.scalar:
                                eng.copy(ot[:], t_[:])
                            else:
                                eng.tensor_copy(ot[:], t_[:])
                            dq.dma_start(
                                out[rr:rr + P, hh:hh + NQ], ot[:])

                    combine(ms, f"2_{S}_{hg}_{co2}", g2_sink,
                            tail=(S == 3 and hg == 3 and co2 == 1))

    nc.compile()
    return nc


_NC = None


def _host_prep(x, wi, wo):
    """Per-expert Strassen operand combos + bf16 casts (host side)."""
    bf = ml_dtypes.bfloat16
    xT = np.ascontiguousarray(np.swapaxes(x, 1, 2))      # [E, H, C]
    w11 = wi[:, :H2, :I2]; w12 = wi[:, :H2, I2:]
    w21 = wi[:, H2:, :I2]; w22 = wi[:, H2:, I2:]
    # G1 lhsT combos, product order M1..M7
    was = [w11 + w22, w12 + w22, w11, w22, w11 + w21, w12 - w11, w21 - w22]
    # pre-tile each combo [1024, 2048] -> [16io*128pp, 8k*128i2] (2KB runs)
    wa = np.stack(
        [np.ascontiguousarray(
            c.reshape(E, K8, P, 16, P)
            .transpose(0, 3, 2, 1, 4).reshape(E, 16 * P, K8 * P))
         for c in was], axis=1).reshape(E, 7 * 16 * P, K8 * P).astype(bf)
    b11 = xT[:, :H2, :C2]; b12 = xT[:, :H2, C2:]
    b21 = xT[:, H2:, :C2]; b22 = xT[:, H2:, C2:]
    xbs = [b11 + b22, b11, b12 - b22, b21 - b11, b22, b11 + b12, b21 + b22]
    xbc = np.concatenate(xbs, axis=1).astype(bf)         # [E, 7*H2, C2]
    # G2 rhs combos of wo quadrants, product order N1..N7
    q11 = wo[:, :I2, :H2]; q12 = wo[:, :I2, H2:]
    q21 = wo[:, I2:, :H2]; q22 = wo[:, I2:, H2:]
    wrs = [q11 + q22, q11, q12 - q22, q21 - q11, q22, q11 + q12, q21 + q22]
    # pre-tile [2048, 1024] -> [4hg*128pp, 16ik*256h2] (8KB runs)
    wrc = np.stack(
        [np.ascontiguousarray(
            c.reshape(E, K16, P, 4, NQ)
            .transpose(0, 3, 2, 1, 4).reshape(E, 4 * P, K16 * NQ))
         for c in wrs], axis=1).reshape(E, 7 * 4 * P, K16 * NQ).astype(bf)
    return wa, xbc, wrc


def kernel(x, wi, wo):
    global _NC
    if _NC is None:
        _NC = _build()
    x = np.asarray(x, dtype=np.float32).reshape(E, C, H)
    wi = np.ascontiguousarray(np.asarray(wi, dtype=np.float32))
    wo = np.ascontiguousarray(np.asarray(wo, dtype=np.float32))
    wa, xbc, wrc = _host_prep(x, wi, wo)
    in_maps = [{"wa": wa[e], "xb": xbc[e], "wr": wrc[e]} for e in range(E)]
    res = run_bass_kernel_spmd(_NC, in_maps, core_ids=list(range(E)))
    o = np.stack([res.results[e]["out"] for e in range(E)])[None]
    return o
